# revision 5
# baseline (speedup 1.0000x reference)
"""Trainium2 Bass kernel for AttentionSR (spatial-reduction attention).

Reference computation (per batch b):
  q = x @ Wq.T                                   [4096, 512] -> heads [8, 4096, 64]
  x_ = conv2x2_stride2(x as NCHW image, Wsr) + bsr   -> [1024, 512]
  x_ = layernorm(x_, g, b)
  k, v = split(x_ @ Wkv.T)                       [8, 1024, 64] each
  out = softmax(q k^T / 8) v                     -> [4096, 512]
  y = out @ Wp.T + bp

Sharding (8 cores): core = 2*batch + query_half. Each core owns one batch's
conv/LN/KV (duplicated across the pair) and 2048 of its 4096 query rows.
No collectives.

On-device layout is channel-major throughout (host passes x transposed), so
the kernel needs no PE transposes:
  conv/q/k projections keep channels on partitions; v is produced token-major
  by swapping the stationary matmul operand; attention scores are computed
  transposed [keys, q]; the softmax denominator comes from a ones column
  appended to v in the attn@v stationary operand; the final division is a
  reciprocal + gpsimd partition-broadcast + multiply on the [64, q] output.
All matmuls run as float32r (fp32 bits, 4x faster PE path, ~1.5e-4 rel err).
LayerNorm is folded into the KV projection: x_scaled = x_raw * rstd plus two
extension rows (s2 = -mu*rstd, ones) against host-extended weights
[ (W*g).T ; W@g ; W@b ].
"""

import numpy as np
from contextlib import ExitStack

import concourse.bass as bass
import concourse.bacc as bacc
import concourse.tile as tile
from concourse import mybir
from concourse.bass_utils import run_bass_kernel_spmd

F32 = mybir.dt.float32
F32R = mybir.dt.float32r
AF = mybir.ActivationFunctionType
ALU = mybir.AluOpType

C = 512          # model dim
NHEAD = 8
DH = 64          # head dim
HS = WS = 64     # image height/width
NTOK = HS * WS   # 4096 tokens per batch
NQ = 2048        # query rows per core
NKV = 1024       # reduced tokens (keys)
B = 4
SCALE = DH ** -0.5
EPS = 1e-5


def _emit(nc, tc, ctx, io):
    xq, xo, w2, wq, wkg, wkg2, wvg, wvg2, wp, bsr_t, bp_t, ones_row, ones_col, yt = io

    pp = ctx.enter_context(tc.tile_pool(name="pp", bufs=4, space="PSUM"))
    pav = ctx.enter_context(tc.tile_pool(name="pav", bufs=2, space="PSUM"))
    persist = ctx.enter_context(tc.tile_pool(name="persist", bufs=1))
    small = ctx.enter_context(tc.tile_pool(name="small", bufs=1))

    # ---- persistent sbuf tensors ----
    qT = [persist.tile([128, NQ], F32R, tag=f"qT{i}", name=f"qT{i}") for i in range(4)]
    kT = [persist.tile([128, NKV], F32R, tag=f"kT{i}", name=f"kT{i}") for i in range(4)]
    v_sb = [persist.tile([128, NHEAD, DH + 1], F32R, tag=f"v{i}", name=f"v{i}") for i in range(8)]
    vout = [persist.tile([128, NQ], F32R, tag=f"vout{i}", name=f"vout{i}") for i in range(4)]
    x_raw = [persist.tile([128, NKV], F32R, tag=f"vout{i}", name=f"xraw{i}") for i in range(4)]

    bsr_sb = small.tile([128, 4], F32)
    nc.sync.dma_start(out=bsr_sb[:], in_=bsr_t)
    bp_sb = small.tile([128, 4], F32)
    nc.sync.dma_start(out=bp_sb[:], in_=bp_t)
    ones_c = small.tile([128, 1], F32R)
    nc.sync.dma_start(out=ones_c[:], in_=ones_col[:, 0:1])
    eps_sb = small.tile([1, 1], F32)
    nc.vector.memset(eps_sb[:], EPS)
    # LN row tensors ([1, N] tiles, base partition 0; values overwritten in place)
    sum_row = small.tile([1, NKV], F32)    # sum -> m
    sq_row = small.tile([1, NKV], F32)     # sumsq -> var -> std -> rstd
    msq_row = small.tile([1, NKV], F32)    # m^2
    rstd_bc = small.tile([128, NKV], F32)
    xs_ext2 = small.tile([2, NKV], F32R)   # row0 = -mu*rstd, row1 = ones (DMA)
    nc.sync.dma_start(out=xs_ext2[1:2, :], in_=ones_row)

    # ================= Phase A: conv + q projection =================
    with tc.tile_pool(name="pA", bufs=1) as pA, \
         tc.tile_pool(name="xstream", bufs=2) as pX:
        w2_sb = pA.tile([128, 16, 512], F32R)
        nc.sync.dma_start(out=w2_sb[:], in_=w2.rearrange("(kk p) o -> p kk o", p=128))
        wq_sb = pA.tile([128, 4, 512], F32R)
        nc.sync.dma_start(out=wq_sb[:], in_=wq.rearrange("(ct p) o -> p ct o", p=128))

        for qtr in range(4):            # quarters: 0,1 = query half; 2,3 = other
            half, lq = qtr // 2, qtr % 2
            src = xq if half == 0 else xo
            xh = pX.tile([128, 4, 1024], F32R, tag="xh", name="xh")
            nc.sync.dma_start(
                out=xh[:],
                in_=src.rearrange("(ct p) t -> p ct t", p=128)[:, :, lq * 1024:(lq + 1) * 1024],
            )

            # conv: tok' chunk [qtr*256, qtr*256+256), 8 i-rows x 32 j
            for ot in range(4):
                ps = pp.tile([128, 512], F32, tag="ps")
                psv = ps[:, 0:256].rearrange("p (a b) -> p a b", a=8)
                for kk in range(16):
                    di, dj, ct = kk // 8, (kk // 4) % 2, kk % 4
                    rhs = bass.AP(
                        tensor=xh[:].tensor,
                        offset=xh[:].offset + ct * 1024 + di * WS + dj,
                        ap=[xh[:].ap[0], [2 * WS, 8], [2, 32]],
                    )
                    nc.tensor.matmul(
                        psv, lhsT=w2_sb[:, kk, ot * 128:(ot + 1) * 128], rhs=rhs,
                        start=(kk == 0), stop=(kk == 15),
                    )
                nc.vector.tensor_scalar_add(
                    x_raw[ot][:, qtr * 256:(qtr + 1) * 256], ps[:, 0:256],
                    bsr_sb[:, ot:ot + 1],
                )

            # q projection for the query half
            if half == 0:
                for ot in range(4):
                    for qc in range(2):
                        ps = pp.tile([128, 512], F32, tag="ps")
                        for ct in range(4):
                            nc.tensor.matmul(
                                ps[:],
                                lhsT=wq_sb[:, ct, ot * 128:(ot + 1) * 128],
                                rhs=xh[:, ct, qc * 512:(qc + 1) * 512],
                                start=(ct == 0), stop=(ct == 3),
                            )
                        qsl = slice(lq * 1024 + qc * 512, lq * 1024 + qc * 512 + 512)
                        nc.vector.tensor_copy(qT[ot][:, qsl], ps[:])

    # ================= Phase B: LN stats + KV projection =================
    with tc.tile_pool(name="pB", bufs=4) as pB, \
         tc.tile_pool(name="pBw", bufs=1) as pBw:
        # squares
        xsq = []
        for ct in range(4):
            t = pB.tile([128, NKV], F32R, tag="xsq", name="xsq")
            nc.scalar.activation(t[:], x_raw[ct][:], AF.Square)
            xsq.append(t)
        # stats: sum and sumsq over channels via ones-matmuls
        for chunk in range(2):
            sl = slice(chunk * 512, (chunk + 1) * 512)
            ps = pp.tile([128, 512], F32, tag="ps")
            for ct in range(4):
                nc.tensor.matmul(ps[0:1, :], lhsT=ones_c[:], rhs=x_raw[ct][:, sl],
                                 start=(ct == 0), stop=(ct == 3))
            nc.vector.tensor_copy(sum_row[0:1, sl], ps[0:1, :])
            ps2 = pp.tile([128, 512], F32, tag="ps")
            for ct in range(4):
                nc.tensor.matmul(ps2[0:1, :], lhsT=ones_c[:], rhs=xsq[ct][:, sl],
                                 start=(ct == 0), stop=(ct == 3))
            nc.vector.tensor_copy(sq_row[0:1, sl], ps2[0:1, :])

        inv_c = 1.0 / C
        nc.vector.tensor_scalar_mul(sum_row[:], sum_row[:], inv_c)        # m
        nc.vector.tensor_mul(msq_row[:], sum_row[:], sum_row[:])          # m^2
        nc.vector.scalar_tensor_tensor(sq_row[:], sq_row[:], inv_c, msq_row[:],
                                       op0=ALU.mult, op1=ALU.subtract)    # var
        nc.scalar.activation(sq_row[:], sq_row[:], AF.Sqrt, bias=eps_sb[:])  # std
        nc.vector.reciprocal(sq_row[:], sq_row[:])                        # rstd
        nc.vector.scalar_tensor_tensor(xs_ext2[0:1, :], sum_row[:], -1.0, sq_row[:],
                                       op0=ALU.mult, op1=ALU.mult)        # s2
        nc.gpsimd.partition_broadcast(rstd_bc[:], sq_row[:])

        # x_scaled = x_raw * rstd  (LN fold; mean/gain/bias live in ext rows/weights)
        xs_ln = []
        for ct in range(4):
            t = pB.tile([128, NKV], F32R, tag="xsq", name="xsq")
            nc.vector.tensor_mul(t[:], x_raw[ct][:].bitcast(F32), rstd_bc[:])
            xs_ln.append(t)

        wkg_sb = pBw.tile([128, 4, 512], F32R)
        nc.sync.dma_start(out=wkg_sb[:], in_=wkg.rearrange("(ct p) o -> p ct o", p=128))
        wkg2_sb = pBw.tile([2, 512], F32R)
        nc.sync.dma_start(out=wkg2_sb[:], in_=wkg2)
        wvg_sb = pBw.tile([128, 4, 512], F32R)
        nc.sync.dma_start(out=wvg_sb[:], in_=wvg.rearrange("(ct p) o -> p ct o", p=128))
        wvg2_sb = pBw.tile([2, 512], F32R)
        nc.sync.dma_start(out=wvg2_sb[:], in_=wvg2)

        # kT[o, tok'] (channel-major keys)
        for ot in range(4):
            for t2 in range(2):
                sl = slice(t2 * 512, (t2 + 1) * 512)
                ps = pp.tile([128, 512], F32, tag="ps")
                for ct in range(4):
                    nc.tensor.matmul(ps[:], lhsT=wkg_sb[:, ct, ot * 128:(ot + 1) * 128],
                                     rhs=xs_ln[ct][:, sl], start=(ct == 0), stop=False)
                nc.tensor.matmul(ps[:], lhsT=wkg2_sb[:, ot * 128:(ot + 1) * 128],
                                 rhs=xs_ext2[:, sl], start=False, stop=True)
                nc.vector.tensor_copy(kT[ot][:, sl], ps[:])

        # v[tok', o] (token-major values) + ones column per head
        for tt in range(8):
            sl = slice(tt * 128, (tt + 1) * 128)
            ps = pp.tile([128, 512], F32, tag="ps")
            for ct in range(4):
                nc.tensor.matmul(ps[:], lhsT=xs_ln[ct][:, sl], rhs=wvg_sb[:, ct, :],
                                 start=(ct == 0), stop=False)
            nc.tensor.matmul(ps[:], lhsT=xs_ext2[:, sl], rhs=wvg2_sb[:],
                             start=False, stop=True)
            nc.vector.tensor_copy(
                v_sb[tt][:, :, 0:DH],
                ps[:].rearrange("p (h d) -> p h d", h=NHEAD),
            )
            nc.sync.dma_start(out=v_sb[tt][:, :, DH:DH + 1], in_=ones_col)

    # ================= Phase C: attention =================
    pexp = ctx.enter_context(tc.tile_pool(name="pexp", bufs=8))
    psig = ctx.enter_context(tc.tile_pool(name="psig", bufs=2))
    for qh in range(2):
        for h in range(8):
            pt, rr = h // 2, (h % 2) * 64
            av = pav.tile([65, 1024], F32, tag="av")
            for kt in range(8):
                for qc in range(2):
                    qsl = slice(qh * 1024 + qc * 512, qh * 1024 + qc * 512 + 512)
                    sc = pp.tile([128, 512], F32, tag="ps")
                    nc.tensor.matmul(
                        sc[:],
                        lhsT=kT[pt][rr:rr + 64, kt * 128:(kt + 1) * 128],
                        rhs=qT[pt][rr:rr + 64, qsl],
                        start=True, stop=True,
                    )
                    ex = pexp.tile([128, 512], F32R, tag="exp")
                    nc.scalar.activation(ex[:], sc[:], AF.Exp, scale=SCALE)
                    nc.tensor.matmul(
                        av[:, qc * 512:(qc + 1) * 512],
                        lhsT=v_sb[kt][:, h, :], rhs=ex[:],
                        start=(kt == 0), stop=(kt == 7),
                    )
            rbc = psig.tile([64, 1024], F32, tag="rbc")
            nc.vector.reciprocal(rbc[0:1, :], av[64:65, :])
            nc.gpsimd.partition_broadcast(rbc[:], rbc[0:1, :])
            nc.vector.tensor_mul(
                vout[pt][rr:rr + 64, qh * 1024:(qh + 1) * 1024],
                av[0:64, :], rbc[:],
            )

        # ---- output projection for this query half (overlaps next half) ----
        if qh == 0:
            wp_sb = persist.tile([128, 4, 512], F32R, tag="wp")
            nc.sync.dma_start(out=wp_sb[:], in_=wp.rearrange("(ct p) o -> p ct o", p=128))
        py = ctx.enter_context(tc.tile_pool(name=f"py{qh}", bufs=3))
        for ot in range(4):
            for qc in range(2):
                qsl = slice(qh * 1024 + qc * 512, qh * 1024 + qc * 512 + 512)
                ps = pp.tile([128, 512], F32, tag="ps")
                for ct in range(4):
                    nc.tensor.matmul(ps[:], lhsT=wp_sb[:, ct, ot * 128:(ot + 1) * 128],
                                     rhs=vout[ct][:, qsl], start=(ct == 0), stop=(ct == 3))
                yt_t = py.tile([128, 512], F32, tag="y")
                nc.vector.tensor_scalar_add(yt_t[:], ps[:], bp_sb[:, ot:ot + 1])
                nc.sync.dma_start(out=yt[ot * 128:(ot + 1) * 128, qsl], in_=yt_t[:])


_CACHE = {}


def _build():
    if "nc" in _CACHE:
        return _CACHE["nc"]
    nc = bacc.Bacc("TRN2", target_bir_lowering=False, debug=False, num_devices=8)
    io = (
        nc.dram_tensor("xq", [C, NQ], F32R, kind="ExternalInput").ap(),
        nc.dram_tensor("xo", [C, NQ], F32R, kind="ExternalInput").ap(),
        nc.dram_tensor("w2", [4 * C, C], F32R, kind="ExternalInput").ap(),
        nc.dram_tensor("wq", [C, C], F32R, kind="ExternalInput").ap(),
        nc.dram_tensor("wkg", [C, C], F32R, kind="ExternalInput").ap(),
        nc.dram_tensor("wkg2", [2, C], F32R, kind="ExternalInput").ap(),
        nc.dram_tensor("wvg", [C, C], F32R, kind="ExternalInput").ap(),
        nc.dram_tensor("wvg2", [2, C], F32R, kind="ExternalInput").ap(),
        nc.dram_tensor("wp", [C, C], F32R, kind="ExternalInput").ap(),
        nc.dram_tensor("bsr_t", [128, 4], F32, kind="ExternalInput").ap(),
        nc.dram_tensor("bp_t", [128, 4], F32, kind="ExternalInput").ap(),
        nc.dram_tensor("ones_row", [1, NKV], F32R, kind="ExternalInput").ap(),
        nc.dram_tensor("ones_col", [128, 8], F32R, kind="ExternalInput").ap(),
        nc.dram_tensor("yt", [C, NQ], F32, kind="ExternalOutput").ap(),
    )
    with tile.TileContext(nc) as tc, ExitStack() as ctx:
        _emit(nc, tc, ctx, io)
    nc.compile()
    _CACHE["nc"] = nc
    return nc


def _prep_inputs(x, Wq, Wkv, Wsr, bsr, ln_g, ln_b, Wp, bp):
    x = np.asarray(x, np.float32)
    Wq = np.asarray(Wq, np.float32)
    Wkv = np.asarray(Wkv, np.float32)
    Wsr = np.asarray(Wsr, np.float32)
    bsr = np.asarray(bsr, np.float32)
    ln_g = np.asarray(ln_g, np.float32)
    ln_b = np.asarray(ln_b, np.float32)
    Wp = np.asarray(Wp, np.float32)
    bp = np.asarray(bp, np.float32)

    w2 = np.ascontiguousarray(Wsr.transpose(2, 3, 1, 0).reshape(4 * C, C))
    wq = np.ascontiguousarray(Wq.T)
    Wk, Wv = Wkv[:C], Wkv[C:]

    def ext(W):
        main = np.ascontiguousarray((W * ln_g[None, :]).T)          # [c, o]
        rows = np.stack([W @ ln_g, W @ ln_b]).astype(np.float32)    # [2, o]
        return main, np.ascontiguousarray(rows)

    wkg, wkg2 = ext(Wk)
    wvg, wvg2 = ext(Wv)
    wp = np.ascontiguousarray(Wp.T)
    bsr_t = np.ascontiguousarray(bsr.reshape(4, 128).T)
    bp_t = np.ascontiguousarray(bp.reshape(4, 128).T)

    shared = dict(w2=w2, wq=wq, wkg=wkg, wkg2=wkg2, wvg=wvg, wvg2=wvg2,
                  wp=wp, bsr_t=bsr_t, bp_t=bp_t,
                  ones_row=np.ones((1, NKV), np.float32),
                  ones_col=np.ones((128, 8), np.float32))
    in_maps = []
    for core in range(8):
        b, half = core // 2, core % 2
        xT = np.ascontiguousarray(x[b].T)     # [C, NTOK]
        m = dict(shared)
        m["xq"] = np.ascontiguousarray(xT[:, half * NQ:(half + 1) * NQ])
        m["xo"] = np.ascontiguousarray(xT[:, (1 - half) * NQ:(2 - half) * NQ])
        in_maps.append(m)
    return in_maps


def kernel(x, H, W, Wq, Wkv, Wsr, bsr, ln_g, ln_b, Wp, bp, _trace=False):
    nc = _build()
    in_maps = _prep_inputs(x, Wq, Wkv, Wsr, bsr, ln_g, ln_b, Wp, bp)
    res = run_bass_kernel_spmd(nc, in_maps, list(range(8)), trace=_trace)
    y = np.empty((B, NTOK, C), np.float32)
    for core in range(8):
        b, half = core // 2, core % 2
        y[b, half * NQ:(half + 1) * NQ, :] = res.results[core]["yt"].T
    kernel._last_result = res
    return y


# revision 9
# speedup vs baseline: 1.4983x; 1.4983x over previous
"""Trainium2 Bass kernel for AttentionSR (spatial-reduction attention).

Reference computation (per batch b):
  q = x @ Wq.T                                   [4096, 512] -> heads [8, 4096, 64]
  x_ = conv2x2_stride2(x as NCHW image, Wsr) + bsr   -> [1024, 512]
  x_ = layernorm(x_, g, b)
  k, v = split(x_ @ Wkv.T)                       [8, 1024, 64] each
  out = softmax(q k^T / 8) v                     -> [4096, 512]
  y = out @ Wp.T + bp

Sharding (8 cores): core = 2*batch + query_half. Each core owns one batch's
conv/LN/KV (duplicated across the pair) and 2048 of its 4096 query rows.
No collectives.

On-device layout is channel-major throughout (host passes x transposed), so
the kernel needs no PE transposes:
  conv/q/k projections keep channels on partitions; v is produced token-major
  by swapping the stationary matmul operand; attention scores are computed
  transposed [keys, q]; the softmax denominator comes from a ones column
  appended to v in the attn@v stationary operand; the final division is a
  reciprocal + gpsimd partition-broadcast + multiply on the [64, q] output.
All matmuls run as float32r (fp32 bits, 4x faster PE path, ~1.5e-4 rel err).
LayerNorm is folded into the KV projection: x_scaled = x_raw * rstd plus two
extension rows (s2 = -mu*rstd, ones) against host-extended weights
[ (W*g).T ; W@g ; W@b ].
"""

import numpy as np
import ml_dtypes
from contextlib import ExitStack

BF = ml_dtypes.bfloat16

import concourse.bass as bass
import concourse.bacc as bacc
import concourse.tile as tile
from concourse import mybir
from concourse.bass_utils import run_bass_kernel_spmd

F32 = mybir.dt.float32
F32R = mybir.dt.float32r
BF16 = mybir.dt.bfloat16
AF = mybir.ActivationFunctionType
ALU = mybir.AluOpType

C = 512          # model dim
NHEAD = 8
DH = 64          # head dim
HS = WS = 64     # image height/width
NTOK = HS * WS   # 4096 tokens per batch
NQ = 2048        # query rows per core
NKV = 1024       # reduced tokens (keys)
B = 4
SCALE = DH ** -0.5
EPS = 1e-5


def _emit(nc, tc, ctx, io, dbg=None):
    xq, xo, w2, wq, wkg, wkg2, wvg, wvg2, wp, bsr_t, bp_t, ones_row, ones_col, ones_c1, yt = io

    pp = ctx.enter_context(tc.tile_pool(name="pp", bufs=4, space="PSUM"))
    pav = ctx.enter_context(tc.tile_pool(name="pav", bufs=2, space="PSUM"))
    persist = ctx.enter_context(tc.tile_pool(name="persist", bufs=1))
    small = ctx.enter_context(tc.tile_pool(name="small", bufs=1))

    # ---- persistent sbuf tensors ----
    qT = [persist.tile([128, NQ], BF16, tag=f"qT{i}", name=f"qT{i}") for i in range(4)]
    kT = [persist.tile([128, NKV], BF16, tag=f"kT{i}", name=f"kT{i}") for i in range(4)]
    v_sb = [persist.tile([128, NHEAD, DH + 1], BF16, tag=f"v{i}", name=f"v{i}") for i in range(8)]
    vout = [persist.tile([128, NQ], BF16, tag=f"vout{i}", name=f"vout{i}") for i in range(4)]
    x_raw = [persist.tile([128, NKV], F32R, tag=f"vout{i}", name=f"xraw{i}") for i in range(4)]

    bsr_sb = small.tile([128, 4], F32)
    nc.sync.dma_start(out=bsr_sb[:], in_=bsr_t)
    bp_sb = small.tile([128, 4], F32)
    nc.sync.dma_start(out=bp_sb[:], in_=bp_t)
    ones_c = small.tile([128, 1], F32R)
    nc.sync.dma_start(out=ones_c[:], in_=ones_c1)
    eps_sb = small.tile([1, 1], F32)
    nc.vector.memset(eps_sb[:], EPS)
    # LN row tensors ([1, N] tiles, base partition 0; values overwritten in place)
    sum_row = small.tile([1, NKV], F32)    # sum -> m
    sq_row = small.tile([1, NKV], F32)     # sumsq -> var -> std -> rstd
    msq_row = small.tile([1, NKV], F32)    # m^2
    rstd_bc = small.tile([128, NKV], F32)
    xs_ext2 = small.tile([2, NKV], BF16)   # row0 = -mu*rstd, row1 = ones (DMA)
    nc.sync.dma_start(out=xs_ext2[1:2, :], in_=ones_row)

    # ================= Phase A: conv + q projection =================
    with tc.tile_pool(name="pA", bufs=1) as pA, \
         tc.tile_pool(name="xstream", bufs=2) as pX:
        w2_sb = pA.tile([128, 16, 512], BF16)
        nc.sync.dma_start(out=w2_sb[:], in_=w2.rearrange("(kk p) o -> p kk o", p=128))
        wq_sb = pA.tile([128, 4, 512], BF16)
        nc.sync.dma_start(out=wq_sb[:], in_=wq.rearrange("(ct p) o -> p ct o", p=128))

        for qtr in range(4):            # quarters: 0,1 = query half; 2,3 = other
            half, lq = qtr // 2, qtr % 2
            src = xq if half == 0 else xo
            xh = pX.tile([128, 4, 1024], BF16, tag="xh", name="xh")
            nc.sync.dma_start(
                out=xh[:],
                in_=src.rearrange("(ct p) t -> p ct t", p=128)[:, :, lq * 1024:(lq + 1) * 1024],
            )

            # conv: tok' chunk [qtr*256, qtr*256+256), 8 i-rows x 32 j
            for ot in range(4):
                ps = pp.tile([128, 512], F32, tag="ps")
                psv = ps[:, 0:256].rearrange("p (a b) -> p a b", a=8)
                for kk in range(16):
                    di, dj, ct = kk // 8, (kk // 4) % 2, kk % 4
                    rhs = bass.AP(
                        tensor=xh[:].tensor,
                        offset=xh[:].offset + ct * 1024 + di * WS + dj,
                        ap=[xh[:].ap[0], [2 * WS, 8], [2, 32]],
                    )
                    nc.tensor.matmul(
                        psv, lhsT=w2_sb[:, kk, ot * 128:(ot + 1) * 128], rhs=rhs,
                        start=(kk == 0), stop=(kk == 15),
                    )
                nc.vector.tensor_scalar_add(
                    x_raw[ot][:, qtr * 256:(qtr + 1) * 256], ps[:, 0:256],
                    bsr_sb[:, ot:ot + 1],
                )

            # q projection for the query half
            if half == 0:
                for ot in range(4):
                    for qc in range(2):
                        ps = pp.tile([128, 512], F32, tag="ps")
                        for ct in range(4):
                            nc.tensor.matmul(
                                ps[:],
                                lhsT=wq_sb[:, ct, ot * 128:(ot + 1) * 128],
                                rhs=xh[:, ct, qc * 512:(qc + 1) * 512],
                                start=(ct == 0), stop=(ct == 3),
                            )
                        qsl = slice(lq * 1024 + qc * 512, lq * 1024 + qc * 512 + 512)
                        nc.vector.tensor_copy(qT[ot][:, qsl], ps[:])

    # ================= Phase B: LN stats + KV projection =================
    with tc.tile_pool(name="pB", bufs=4) as pB, \
         tc.tile_pool(name="pBw", bufs=1) as pBw:
        # squares
        xsq = []
        for ct in range(4):
            t = pB.tile([128, NKV], F32R, tag="xsq", name="xsq")
            nc.scalar.activation(t[:], x_raw[ct][:], AF.Square)
            xsq.append(t)
        # stats: sum and sumsq over channels via ones-matmuls
        for chunk in range(2):
            sl = slice(chunk * 512, (chunk + 1) * 512)
            ps = pp.tile([128, 512], F32, tag="ps")
            for ct in range(4):
                nc.tensor.matmul(ps[0:1, :], lhsT=ones_c[:], rhs=x_raw[ct][:, sl],
                                 start=(ct == 0), stop=(ct == 3))
            nc.vector.tensor_copy(sum_row[0:1, sl], ps[0:1, :])
            ps2 = pp.tile([128, 512], F32, tag="ps")
            for ct in range(4):
                nc.tensor.matmul(ps2[0:1, :], lhsT=ones_c[:], rhs=xsq[ct][:, sl],
                                 start=(ct == 0), stop=(ct == 3))
            nc.vector.tensor_copy(sq_row[0:1, sl], ps2[0:1, :])

        inv_c = 1.0 / C
        nc.vector.tensor_scalar_mul(sum_row[:], sum_row[:], inv_c)        # m
        nc.vector.tensor_mul(msq_row[:], sum_row[:], sum_row[:])          # m^2
        nc.vector.scalar_tensor_tensor(sq_row[:], sq_row[:], inv_c, msq_row[:],
                                       op0=ALU.mult, op1=ALU.subtract)    # var
        nc.scalar.activation(sq_row[:], sq_row[:], AF.Sqrt, bias=eps_sb[:])  # std
        nc.vector.reciprocal(sq_row[:], sq_row[:])                        # rstd
        nc.vector.scalar_tensor_tensor(xs_ext2[0:1, :], sum_row[:], -1.0, sq_row[:],
                                       op0=ALU.mult, op1=ALU.mult)        # s2
        nc.gpsimd.partition_broadcast(rstd_bc[:], sq_row[:])

        # x_scaled = x_raw * rstd  (LN fold; mean/gain/bias live in ext rows/weights)
        xs_ln = []
        for ct in range(4):
            t = pB.tile([128, NKV], BF16, tag="xsq", name="xsq")
            nc.vector.tensor_mul(t[:], x_raw[ct][:].bitcast(F32), rstd_bc[:])
            xs_ln.append(t)

        wkg_sb = pBw.tile([128, 4, 512], BF16)
        nc.sync.dma_start(out=wkg_sb[:], in_=wkg.rearrange("(ct p) o -> p ct o", p=128))
        wkg2_sb = pBw.tile([2, 512], BF16)
        nc.sync.dma_start(out=wkg2_sb[:], in_=wkg2)
        wvg_sb = pBw.tile([128, 4, 512], BF16)
        nc.sync.dma_start(out=wvg_sb[:], in_=wvg.rearrange("(ct p) o -> p ct o", p=128))
        wvg2_sb = pBw.tile([2, 512], BF16)
        nc.sync.dma_start(out=wvg2_sb[:], in_=wvg2)

        # kT[o, tok'] (channel-major keys)
        for ot in range(4):
            for t2 in range(2):
                sl = slice(t2 * 512, (t2 + 1) * 512)
                ps = pp.tile([128, 512], F32, tag="ps")
                for ct in range(4):
                    nc.tensor.matmul(ps[:], lhsT=wkg_sb[:, ct, ot * 128:(ot + 1) * 128],
                                     rhs=xs_ln[ct][:, sl], start=(ct == 0), stop=False)
                nc.tensor.matmul(ps[:], lhsT=wkg2_sb[:, ot * 128:(ot + 1) * 128],
                                 rhs=xs_ext2[:, sl], start=False, stop=True)
                nc.vector.tensor_copy(kT[ot][:, sl], ps[:])

        # v[tok', o] (token-major values) + ones column per head
        for tt in range(8):
            sl = slice(tt * 128, (tt + 1) * 128)
            ps = pp.tile([128, 512], F32, tag="ps")
            for ct in range(4):
                nc.tensor.matmul(ps[:], lhsT=xs_ln[ct][:, sl], rhs=wvg_sb[:, ct, :],
                                 start=(ct == 0), stop=False)
            nc.tensor.matmul(ps[:], lhsT=xs_ext2[:, sl], rhs=wvg2_sb[:],
                             start=False, stop=True)
            nc.vector.tensor_copy(
                v_sb[tt][:, :, 0:DH],
                ps[:].rearrange("p (h d) -> p h d", h=NHEAD),
            )
            nc.sync.dma_start(out=v_sb[tt][:, :, DH:DH + 1], in_=ones_col)

    # ================= Phase C: attention =================
    pexp = ctx.enter_context(tc.tile_pool(name="pexp", bufs=8))
    psig = ctx.enter_context(tc.tile_pool(name="psig", bufs=2))
    for qh in range(2):
        for h in range(8):
            pt, rr = h // 2, (h % 2) * 64
            av = pav.tile([65, 1024], F32, tag="av")
            for kt in range(8):
                for qc in range(2):
                    qsl = slice(qh * 1024 + qc * 512, qh * 1024 + qc * 512 + 512)
                    sc = pp.tile([128, 512], F32, tag="ps")
                    nc.tensor.matmul(
                        sc[:],
                        lhsT=kT[pt][rr:rr + 64, kt * 128:(kt + 1) * 128],
                        rhs=qT[pt][rr:rr + 64, qsl],
                        start=True, stop=True,
                    )
                    ex = pexp.tile([128, 512], BF16, tag="exp")
                    nc.scalar.activation(ex[:], sc[:], AF.Exp, scale=SCALE)
                    nc.tensor.matmul(
                        av[:, qc * 512:(qc + 1) * 512],
                        lhsT=v_sb[kt][:, h, :], rhs=ex[:],
                        start=(kt == 0), stop=(kt == 7),
                    )
            sig = psig.tile([1, 1024], F32, tag="sig")
            nc.vector.tensor_copy(sig[:], av[64:65, :])
            rbc = psig.tile([64, 1024], F32, tag="rbc")
            nc.vector.reciprocal_approx_fast(out=rbc[0:1, :], in_=sig[:])
            nc.gpsimd.partition_broadcast(rbc[:], rbc[0:1, :])
            nc.vector.tensor_mul(
                vout[pt][rr:rr + 64, qh * 1024:(qh + 1) * 1024],
                av[0:64, :], rbc[:],
            )

        # ---- output projection for this query half (overlaps next half) ----
        if qh == 0:
            wp_sb = persist.tile([128, 4, 512], BF16, tag="wp")
            nc.sync.dma_start(out=wp_sb[:], in_=wp.rearrange("(ct p) o -> p ct o", p=128))
        py = ctx.enter_context(tc.tile_pool(name=f"py{qh}", bufs=3))
        for ot in range(4):
            for qc in range(2):
                qsl = slice(qh * 1024 + qc * 512, qh * 1024 + qc * 512 + 512)
                ps = pp.tile([128, 512], F32, tag="ps")
                for ct in range(4):
                    nc.tensor.matmul(ps[:], lhsT=wp_sb[:, ct, ot * 128:(ot + 1) * 128],
                                     rhs=vout[ct][:, qsl], start=(ct == 0), stop=(ct == 3))
                yt_t = py.tile([128, 512], F32, tag="y")
                nc.vector.tensor_scalar_add(yt_t[:], ps[:], bp_sb[:, ot:ot + 1])
                nc.sync.dma_start(out=yt[ot * 128:(ot + 1) * 128, qsl], in_=yt_t[:])

    if dbg is not None:
        for i in range(4):
            nc.sync.dma_start(out=dbg[f"dbg_xraw{i}"], in_=x_raw[i][:].bitcast(F32))
            nc.sync.dma_start(out=dbg[f"dbg_qT{i}"], in_=qT[i][:])
            nc.sync.dma_start(out=dbg[f"dbg_kT{i}"], in_=kT[i][:])
        for i in range(8):
            nc.sync.dma_start(out=dbg[f"dbg_v{i}"], in_=v_sb[i][:])


_CACHE = {}


def _build(debug=False):
    key = ("nc", debug)
    if key in _CACHE:
        return _CACHE[key]
    nc = bacc.Bacc("TRN2", target_bir_lowering=False, debug=False, num_devices=8)
    io = (
        nc.dram_tensor("xq", [C, NQ], BF16, kind="ExternalInput").ap(),
        nc.dram_tensor("xo", [C, NQ], BF16, kind="ExternalInput").ap(),
        nc.dram_tensor("w2", [4 * C, C], BF16, kind="ExternalInput").ap(),
        nc.dram_tensor("wq", [C, C], BF16, kind="ExternalInput").ap(),
        nc.dram_tensor("wkg", [C, C], BF16, kind="ExternalInput").ap(),
        nc.dram_tensor("wkg2", [2, C], BF16, kind="ExternalInput").ap(),
        nc.dram_tensor("wvg", [C, C], BF16, kind="ExternalInput").ap(),
        nc.dram_tensor("wvg2", [2, C], BF16, kind="ExternalInput").ap(),
        nc.dram_tensor("wp", [C, C], BF16, kind="ExternalInput").ap(),
        nc.dram_tensor("bsr_t", [128, 4], F32, kind="ExternalInput").ap(),
        nc.dram_tensor("bp_t", [128, 4], F32, kind="ExternalInput").ap(),
        nc.dram_tensor("ones_row", [1, NKV], BF16, kind="ExternalInput").ap(),
        nc.dram_tensor("ones_col", [128, 8], BF16, kind="ExternalInput").ap(),
        nc.dram_tensor("ones_c1", [128, 1], F32R, kind="ExternalInput").ap(),
        nc.dram_tensor("yt", [C, NQ], F32, kind="ExternalOutput").ap(),
    )
    dbg = None
    if debug:
        dbg = {}
        for i in range(4):
            dbg[f"dbg_xraw{i}"] = nc.dram_tensor(f"dbg_xraw{i}", [128, NKV], F32, kind="ExternalOutput").ap()
            dbg[f"dbg_qT{i}"] = nc.dram_tensor(f"dbg_qT{i}", [128, NQ], BF16, kind="ExternalOutput").ap()
            dbg[f"dbg_kT{i}"] = nc.dram_tensor(f"dbg_kT{i}", [128, NKV], BF16, kind="ExternalOutput").ap()
        for i in range(8):
            dbg[f"dbg_v{i}"] = nc.dram_tensor(f"dbg_v{i}", [128, NHEAD, DH + 1], BF16, kind="ExternalOutput").ap()
    with tile.TileContext(nc) as tc, ExitStack() as ctx:
        _emit(nc, tc, ctx, io, dbg)
    nc.compile()
    _CACHE[key] = nc
    return nc


def _prep_inputs(x, Wq, Wkv, Wsr, bsr, ln_g, ln_b, Wp, bp):
    x = np.asarray(x, np.float32)
    Wq = np.asarray(Wq, np.float32)
    Wkv = np.asarray(Wkv, np.float32)
    Wsr = np.asarray(Wsr, np.float32)
    bsr = np.asarray(bsr, np.float32)
    ln_g = np.asarray(ln_g, np.float32)
    ln_b = np.asarray(ln_b, np.float32)
    Wp = np.asarray(Wp, np.float32)
    bp = np.asarray(bp, np.float32)

    w2 = np.ascontiguousarray(Wsr.transpose(2, 3, 1, 0).reshape(4 * C, C).astype(BF))
    wq = np.ascontiguousarray(Wq.T.astype(BF))
    Wk, Wv = Wkv[:C], Wkv[C:]

    def ext(W):
        main = np.ascontiguousarray((W * ln_g[None, :]).T.astype(BF))   # [c, o]
        rows = np.stack([W @ ln_g, W @ ln_b]).astype(BF)                # [2, o]
        return main, np.ascontiguousarray(rows)

    wkg, wkg2 = ext(Wk)
    wvg, wvg2 = ext(Wv)
    wp = np.ascontiguousarray(Wp.T.astype(BF))
    bsr_t = np.ascontiguousarray(bsr.reshape(4, 128).T)
    bp_t = np.ascontiguousarray(bp.reshape(4, 128).T)

    shared = dict(w2=w2, wq=wq, wkg=wkg, wkg2=wkg2, wvg=wvg, wvg2=wvg2,
                  wp=wp, bsr_t=bsr_t, bp_t=bp_t,
                  ones_row=np.ones((1, NKV), BF),
                  ones_col=np.ones((128, 8), BF),
                  ones_c1=np.ones((128, 1), np.float32))
    in_maps = []
    for core in range(8):
        b, half = core // 2, core % 2
        xT = x[b].T.astype(BF)                # [C, NTOK]
        m = dict(shared)
        m["xq"] = np.ascontiguousarray(xT[:, half * NQ:(half + 1) * NQ])
        m["xo"] = np.ascontiguousarray(xT[:, (1 - half) * NQ:(2 - half) * NQ])
        in_maps.append(m)
    return in_maps


def kernel(x, H, W, Wq, Wkv, Wsr, bsr, ln_g, ln_b, Wp, bp, _trace=False, _debug=False):
    nc = _build(debug=_debug)
    in_maps = _prep_inputs(x, Wq, Wkv, Wsr, bsr, ln_g, ln_b, Wp, bp)
    res = run_bass_kernel_spmd(nc, in_maps, list(range(8)), trace=_trace)
    y = np.empty((B, NTOK, C), np.float32)
    for core in range(8):
        b, half = core // 2, core % 2
        y[b, half * NQ:(half + 1) * NQ, :] = res.results[core]["yt"].T
    kernel._last_result = res
    if _debug:
        kernel._debug_out = {k: np.asarray(v) for k, v in res.results[0].items() if k.startswith("dbg_")}
    return y
